# revision 57
# baseline (speedup 1.0000x reference)
"""Trainium2 Bass kernel for an 8-head attention layer + FFN (B=2, S=2048,
D=1024, DQK=128, DFF=4096), distributed over 8 NeuronCores.

Sharding: head-parallel attention (1 head per core), token-parallel FFN.
Each core's FFN tokens are rows [c*256, (c+1)*256) of BOTH batches (512
total). Attention outputs for batch b are stored group-major into
cc_in[b] = [8 groups, D, 256]; ONE ReduceScatter per batch then delivers
each core its own fully-reduced 256-token chunk directly — no AllToAll.
RS_0 fires at 50% attention progress (hidden); RS_1's exposure is hidden
under the first half of FFN1, which only needs batch-0 tokens.

DMAs are batched aggressively (multi-dim access patterns, one DMA per
x-chunk / chunk-store / 4-wide weight group) since each HWDGE DMA costs
~600ns of sequencer + DGE time. All DMAs issue from the two HWDGE
queues (sync=SP carries stores+bf16-x, scalar=Act carries fp8-x+weights).

Precision plan (validated vs f64 reference in prec_sim.py, ~4e-3 max rel
err vs the 2e-2 gate):
  - q/k projections bf16, qT/kT stored bf16, scores f32 PSUM
  - v projection fp8(e4m3) DoubleRow for all tokens + an extra bf16
    projection for tokens < 256 (first 256 softmax rows are too short to
    average out fp8 noise)
  - softmax: exp in f32->bf16 (rows 0..255) / fp8 (rest); denominators
    from ones-matmuls over the SAME quantized e tiles; no max-subtraction
    (scores are O(1))
  - attnV: bf16 for rows 0..255, fp8 DoubleRow elsewhere
  - FFN bf16 weights/activations, f32 accumulate + f32 residuals
  - collective payload bf16

Score rows are processed in chunks of [256, 256, 512, 512, 512] rows per
batch; each chunk is emitted right after the 512-token projection group
that completes its causal window, so collectives and DMAs hide under
matmul work. DoubleRow tiles carry a k-pair axis (pair outermost, M=128):
x8 [128, 2, 512], v8 [128, 2, D], e8 [128, 2, 512].
"""
import sys

sys.path.insert(0, "/opt/trn_rl_repo")
import numpy as np
import ml_dtypes

B, S, D, H, DQK, DFF = 2, 2048, 1024, 8, 128, 4096
P = 128
TOKC = 256               # tokens per core per batch (FFN sharding)
NG = S // TOKC           # 8 groups per batch
NCORES = 8
NT = S // P              # 16 t-blocks
ND = D // P              # 8 d-blocks
ND2 = ND // 2            # 4 d-block pairs (DoubleRow)
NF = DFF // P            # 32 f-blocks
NVB = 2                  # t-blocks with a bf16 v copy (short rows)
SCALE = 1.0 / float(np.sqrt(DQK))
FFN_BF16 = True          # kept for test.py compat
# (row_start, width) of score-row chunks per batch; chunk 0 runs the bf16
# path. 512-wide chunks store into two 256-token groups.
CHUNKS = [(0, 256), (256, 256), (512, 512), (1024, 512), (1536, 512)]

F8 = ml_dtypes.float8_e4m3
BF = ml_dtypes.bfloat16


def _mask_schedule(mask):
    """Classify each (t-block, chunk) tile of the score matrix.

    Returns (sched, tiles): sched[ci] = (use_bf, entries) where entries is
    a list of (bt, (key, idx) | None); fully-masked tiles are dropped.
    tiles[key] is a list of [P, w] 0/1 fp32 tiles (layout [t, s]) to
    multiply into exp(s); key in {"b256", "f256", "f512"}.
    """
    mask = np.asarray(mask, dtype=bool)
    sched = []
    tiles = {"b256": [], "f256": [], "f512": []}
    uniq = {}
    for ci, (r0, w) in enumerate(CHUNKS):
        entries = []
        for bt in range(NT):
            sub = mask[r0:r0 + w, bt * P:(bt + 1) * P]  # [s, t]
            if sub.all():
                continue
            entries.append((bt, None if not sub.any() else sub))
        bts = [bt for bt, _ in entries]
        use_bf = (ci == 0 or len(bts) % 2 or bts != list(range(len(bts))))
        if use_bf:
            assert w == 256 and all(bt < NVB for bt in bts), \
                "bf16 fallback path requires all entries in t-blocks 0..1"
        key = ("b" if use_bf else "f") + str(w)
        resolved = []
        for bt, sub in entries:
            if sub is None:
                resolved.append((bt, None))
                continue
            tileT = np.where(sub.T, np.float32(0.0), np.float32(1.0)).copy()
            k2 = (key, tileT.tobytes())
            if k2 not in uniq:
                uniq[k2] = len(tiles[key])
                tiles[key].append(tileT)
            resolved.append((bt, (key, uniq[k2])))
        sched.append((use_bf, resolved))
    return sched, tiles


def _build(sched, nm, collective=True, reps=1):
    """nm: dict key->tile count for the three mask-tile tensors."""
    import concourse.mybir as mybir
    import concourse.tile as tile
    from concourse import bacc

    F32 = mybir.dt.float32
    F32R = mybir.dt.float32r
    BF16 = mybir.dt.bfloat16
    FP8 = mybir.dt.float8e4
    AF = mybir.ActivationFunctionType
    OP = mybir.AluOpType
    DR = mybir.MatmulPerfMode.DoubleRow

    nc = bacc.Bacc("TRN2", target_bir_lowering=False, debug=False,
                   num_devices=NCORES)

    xTb_in = nc.dram_tensor("xTb", [B, 4, P, ND, 512], BF16,
                            kind="ExternalInput")
    x8_in = nc.dram_tensor("x8", [B, 4, P, ND2, 2, 512], FP8,
                           kind="ExternalInput")
    wqT_in = nc.dram_tensor("wqT", [D, DQK], BF16, kind="ExternalInput")
    wkT_in = nc.dram_tensor("wkT", [D, DQK], BF16, kind="ExternalInput")
    wv8_in = nc.dram_tensor("wv8", [P, ND2, 2, D], FP8, kind="ExternalInput")
    wvTb_in = nc.dram_tensor("wvTb", [D, D], BF16, kind="ExternalInput")
    w1b_in = nc.dram_tensor("w1b", [ND, P, DFF], BF16, kind="ExternalInput")
    w2b_in = nc.dram_tensor("w2b", [NF, P, D], BF16, kind="ExternalInput")
    b1_in = nc.dram_tensor("b1c", [P, NF], F32, kind="ExternalInput")
    b2_in = nc.dram_tensor("b2c", [P, ND], F32, kind="ExternalInput")
    nb2 = max(nm.get("b256", 0), 1)
    nf2 = max(nm.get("f256", 0), 1)
    nf5 = max(nm.get("f512", 0), 1)
    mb2_in = nc.dram_tensor("mb256", [nb2, P, 256], BF16,
                            kind="ExternalInput")
    mf2_in = nc.dram_tensor("mf256", [nf2, P, 256], FP8,
                            kind="ExternalInput")
    mf5_in = nc.dram_tensor("mf512", [nf5, P, 512], FP8,
                            kind="ExternalInput")
    onecb_in = nc.dram_tensor("onecb", [P, P], BF16, kind="ExternalInput")
    one8_in = nc.dram_tensor("one8", [P, 2, P], FP8, kind="ExternalInput")
    xTg_in = nc.dram_tensor("xTg", [D, 2 * TOKC], BF16,
                        kind="ExternalInput")
    outT = nc.dram_tensor("outT", [D, 2 * TOKC], BF16, kind="ExternalOutput")

    wq_r = wqT_in.rearrange("(o p) e -> p o e", p=P)
    wk_r = wkT_in.rearrange("(o p) e -> p o e", p=P)
    wvb_r = wvTb_in.rearrange("(o p) d -> p o d", p=P)
    w1_r = w1b_in.rearrange("o p f -> p o f")
    w2_r = w2b_in.rearrange("f p d -> p f d")
    mb2_r = mb2_in.rearrange("n p w -> p n w")
    mf2_r = mf2_in.rearrange("n p w -> p n w")
    mf5_r = mf5_in.rearrange("n p w -> p n w")
    xTg_r = xTg_in.rearrange("(o p) t -> p o t", p=P)
    outT_r = outT.rearrange("(h o p) t -> h p o t", h=2, p=P)

    with tile.TileContext(nc) as tc:
        with (
            tc.tile_pool(name="consts", bufs=1) as consts,
            tc.tile_pool(name="w2p", bufs=4) as w2p,
            tc.tile_pool(name="dram", bufs=1, space="DRAM") as dram,
        ):
            # const tiles are allocated here but their load DMAs are emitted
            # inside emit_body right after the first x loads, ordered by
            # first use, so the critical first projection isn't queued
            # behind ~6MB of constants on the shared HWDGE.
            ones_cb = consts.tile([P, P], BF16, tag="onecb")
            ones8 = consts.tile([P, 2, P], FP8, tag="one8")
            b1_sb = consts.tile([P, NF], F32, tag="b1")
            b2_sb = consts.tile([P, ND], F32, tag="b2")
            mb2_sb = consts.tile([P, nb2, 256], BF16, tag="mb2")
            mf2_sb = consts.tile([P, nf2, 256], FP8, tag="mf2")
            mf5_sb = consts.tile([P, nf5, 512], FP8, tag="mf5")
            msk_sb = {"b256": mb2_sb, "f256": mf2_sb, "f512": mf5_sb}
            wq_all = consts.tile([P, ND, DQK], BF16, tag="wq", name="wq")
            wk_all = consts.tile([P, ND, DQK], BF16, tag="wk", name="wk")
            wv8_all = consts.tile([P, ND2, 2, D], FP8, tag="wv8", name="wv8")
            wvb_all = consts.tile([P, ND, D], BF16, tag="wvb", name="wvb")
            w1_t = [consts.tile([P, ND2, DFF], BF16, tag=f"w1_{i}",
                                name=f"w1_{i}") for i in range(2)]
            # batch-0 FFN inputs, prepared during attention (gpsimd SWDGE
            # keeps the loads off the two busy HWDGE queues) so FFN1
            # half-0 can start the instant attention ends and hide RS_1
            # under matmul work
            xgh0 = consts.tile([P, ND, TOKC], BF16, tag="xgh0")
            co0 = consts.tile([P, ND, TOKC], BF16, tag="co0")
            r1bh0 = [consts.tile([P, TOKC], BF16, tag=f"r1bh0_{do}",
                                 name=f"r1bh0_{do}") for do in range(ND)]
            w1_loaded = []
            consts_loaded = []

            def emit_const_loads():
                nc.sync.dma_start(ones_cb[:], onecb_in[:])
                nc.scalar.dma_start(mb2_sb[:], mb2_r[:])
                nc.sync.dma_start(mf2_sb[:], mf2_r[:])
                nc.scalar.dma_start(mf5_sb[:], mf5_r[:])
                nc.sync.dma_start(ones8[:], one8_in[:])
                nc.scalar.dma_start(b1_sb[:], b1_in[:])
                nc.sync.dma_start(b2_sb[:], b2_in[:])
                nc.scalar.dma_start(wvb_all[:], wvb_r[:])
                nc.sync.dma_start(wv8_all[:], wv8_in[:])

            def emit_body():
                cc_in = dram.tile([B, NG, D, TOKC], BF16, tag="cc_in",
                                  name="cc_in")
                cc_out = dram.tile([B, D, TOKC], BF16, tag="cc_out",
                                   name="cc_out")
                cc_st1 = cc_in.rearrange("b g (o p) s -> b g p o s", p=P)
                cc_ld = cc_out.rearrange("b (o p) s -> b p o s", p=P)

                # ---------------- attention (head-parallel) ----------------
                with (
                    tc.tile_pool(name="xt", bufs=2) as xtp,
                    tc.tile_pool(name="qk", bufs=1) as qkp,
                    tc.tile_pool(name="vp", bufs=1) as vp,
                    tc.tile_pool(name="ep", bufs=10) as ep,
                    tc.tile_pool(name="ebp", bufs=4) as ebp,
                    tc.tile_pool(name="rbp", bufs=2) as rbp,
                    tc.tile_pool(name="aop", bufs=3) as aop,
                    tc.tile_pool(name="ps_pr", bufs=2, space="PSUM") as ps_pr,
                    tc.tile_pool(name="ps_sc", bufs=2, space="PSUM") as ps_sc,
                    tc.tile_pool(name="ps_sum", bufs=1, space="PSUM") as ps_sum,
                    tc.tile_pool(name="ps_at", bufs=3, space="PSUM") as ps_at,
                ):
                    nc.gpsimd.dma_start(xgh0[:], xTg_r[:, :, :TOKC])
                    for b in range(B):
                        qT_t = [qkp.tile([P, 512], BF16, tag=f"qT{t}",
                                         name=f"qT{t}") for t in range(4)]
                        kT_t = [qkp.tile([P, 512], BF16, tag=f"kT{t}",
                                         name=f"kT{t}") for t in range(4)]
                        v8_t = [vp.tile([P, 2, D], FP8, tag=f"v8_{m}",
                                        name=f"v8_{m}")
                                for m in range(NT // 2)]
                        vb_t = [vp.tile([P, D], BF16, tag=f"vb{ti}",
                                        name=f"vb{ti}") for ti in range(NVB)]

                        def emit_chunk(ci):
                            r0, w = CHUNKS[ci]
                            use_bf, entries = sched[ci]
                            if not entries:
                                return
                            tq = r0 // 512
                            qsl = slice(r0 % 512, r0 % 512 + w)
                            sums = ps_sum.tile([P, 512], F32, tag="sum")
                            ao_t = [aop.tile([P, ND, TOKC], BF16, tag="ao",
                                             name=f"ao{gi}")
                                    for gi in range(w // TOKC)]
                            e_sb = {}
                            e_pr = []
                            npair = len(entries) // 2
                            if not use_bf:
                                e_pr = [ep.tile([P, 2, 512], FP8, tag="e8",
                                                name=f"e8_{m}")
                                        for m in range(npair)]
                            # scores, exp, and the ones-matmul denominators
                            # interleaved so the sums never wait on the
                            # full exp backlog
                            for i, (bt, mi) in enumerate(entries):
                                sp = ps_sc.tile([P, 512], F32, tag="sc")
                                nc.tensor.matmul(
                                    sp[:, :w],
                                    kT_t[bt // 4][:, (bt % 4) * P:
                                                  (bt % 4 + 1) * P],
                                    qT_t[tq][:, qsl],
                                    start=True, stop=True)
                                if use_bf:
                                    eb = ebp.tile([P, 256], BF16, tag="eb")
                                    e_sb[bt] = eb
                                    ev = eb[:]
                                else:
                                    ev = e_pr[i // 2][:, i % 2, :w]
                                nc.scalar.activation(ev, sp[:, :w],
                                                     AF.Exp, scale=SCALE)
                                if mi is not None:
                                    nc.vector.tensor_tensor(
                                        ev, ev, msk_sb[mi[0]][:, mi[1], :],
                                        OP.mult)
                                if use_bf:
                                    nc.tensor.matmul(
                                        sums[:, :w], ones_cb[:], ev,
                                        start=(i == 0),
                                        stop=(i == len(entries) - 1))
                                elif i % 2 == 1:
                                    nc.tensor.matmul(
                                        sums[:, :w], ones8[:],
                                        e_pr[i // 2][:, :, :w],
                                        start=(i == 1),
                                        stop=(i == len(entries) - 1),
                                        perf_mode=DR)
                            rb = rbp.tile([P, 512], F32R, tag="rb")
                            with nc.allow_low_precision(
                                    reason="softmax 1/sum in f32r"):
                                nc.vector.reciprocal(rb[:, :w], sums[:, :w])
                            # attnV pair-outer over 4-oc halves: each mm
                            # only needs the next exp pair, so PE chews
                            # through attnV while later exps still run
                            for ocs in ([0], [1], [2], [3], [4], [5],
                                        [6], [7]):
                                aps = {oc: ps_at.tile([P, 512], F32,
                                                      tag="at",
                                                      name=f"at{oc}")
                                       for oc in ocs}
                                if use_bf:
                                    for i, (bt, _mi) in enumerate(entries):
                                        for oc in ocs:
                                            nc.tensor.matmul(
                                                aps[oc][:, :w],
                                                vb_t[bt][:, oc * P:
                                                         (oc + 1) * P],
                                                e_sb[bt][:],
                                                start=(i == 0),
                                                stop=(i == len(entries)
                                                      - 1))
                                else:
                                    for m in range(npair):
                                        for oc in ocs:
                                            nc.tensor.matmul(
                                                aps[oc][:, :w],
                                                v8_t[m][:, :, oc * P:
                                                        (oc + 1) * P],
                                                e_pr[m][:, :, :w],
                                                start=(m == 0),
                                                stop=(m == npair - 1),
                                                perf_mode=DR)
                                for oc in ocs:
                                    for gi in range(w // TOKC):
                                        gs = slice(gi * TOKC,
                                                   (gi + 1) * TOKC)
                                        nc.vector.tensor_tensor(
                                            ao_t[gi][:, oc], aps[oc][:, gs],
                                            rb[:, gs], OP.mult)
                            # stores go through gpsimd SWDGE: the Pool
                            # queue is otherwise idle, and a store waiting
                            # on attention output would head-of-line block
                            # the x loads behind it on a HWDGE queue
                            g0 = r0 // TOKC
                            for gi in range(w // TOKC):
                                nc.gpsimd.dma_start(cc_st1[b, g0 + gi],
                                                    ao_t[gi][:])

                        for tch in range(4):  # 512-token chunks of S
                            if not consts_loaded:
                                # tiny q/k weights first so the first
                                # projection isn't queued behind x
                                nc.sync.dma_start(wq_all[:], wq_r[:])
                                nc.scalar.dma_start(wk_all[:], wk_r[:])
                            xb_all = xtp.tile([P, ND, 512], BF16, tag="xb")
                            nc.sync.dma_start(xb_all[:], xTb_in[b, tch])
                            x8_all = xtp.tile([P, ND2, 2, 512], FP8,
                                              tag="x8")
                            nc.scalar.dma_start(x8_all[:], x8_in[b, tch])
                            if not consts_loaded:
                                consts_loaded.append(True)
                                emit_const_loads()
                            qps = ps_pr.tile([P, 512], F32, tag="pr")
                            for do in range(ND):
                                nc.tensor.matmul(qps[:], wq_all[:, do],
                                                 xb_all[:, do],
                                                 start=(do == 0),
                                                 stop=(do == ND - 1))
                            nc.vector.tensor_copy(qT_t[tch][:], qps[:])
                            kps = ps_pr.tile([P, 512], F32, tag="pr")
                            for do in range(ND):
                                nc.tensor.matmul(kps[:], wk_all[:, do],
                                                 xb_all[:, do],
                                                 start=(do == 0),
                                                 stop=(do == ND - 1))
                            nc.vector.tensor_copy(kT_t[tch][:], kps[:])
                            for ti in range(4):  # t-blocks within this chunk
                                to = tch * 4 + ti
                                tsl = slice(ti * P, (ti + 1) * P)
                                for oc in range(2):
                                    osl = slice(oc * 512, (oc + 1) * 512)
                                    vps = ps_pr.tile([P, 512], F32, tag="pr")
                                    for dp in range(ND2):
                                        nc.tensor.matmul(
                                            vps[:], x8_all[:, dp, :, tsl],
                                            wv8_all[:, dp, :, osl],
                                            start=(dp == 0),
                                            stop=(dp == ND2 - 1),
                                            perf_mode=DR)
                                    nc.vector.tensor_copy(
                                        v8_t[to // 2][:, to % 2, osl],
                                        vps[:])
                                    if tch == 0 and ti < NVB:
                                        vbs = ps_pr.tile([P, 512], F32,
                                                         tag="pr")
                                        for do in range(ND):
                                            nc.tensor.matmul(
                                                vbs[:],
                                                xb_all[:, do, tsl],
                                                wvb_all[:, do, osl],
                                                start=(do == 0),
                                                stop=(do == ND - 1))
                                        nc.scalar.copy(vb_t[ti][:, osl],
                                                       vbs[:])
                            # w1 streams in 1MB pieces (2 per 512-token
                            # chunk) so it never monopolizes the DMA
                            # engines ahead of the latency-critical x loads
                            if not (b == 0 and tch == 0) and \
                                    len(w1_loaded) < ND:
                                n = 2 if len(w1_loaded) < 2 else 1
                                for _ in range(n):
                                    i = len(w1_loaded)
                                    w1_loaded.append(True)
                                    (nc.scalar, nc.sync)[i % 2].dma_start(
                                        w1_t[i // 4][:, i % 4:i % 4 + 1, :],
                                        w1_r[:, i:i + 1, :])
                            if tch == 0:
                                emit_chunk(0)
                                emit_chunk(1)
                            else:
                                emit_chunk(tch + 1)
                        if collective:
                            nc.gpsimd.collective_compute(
                                "ReduceScatter",
                                mybir.AluOpType.add,
                                replica_groups=[list(range(NCORES))],
                                ins=[cc_in[b].opt()],
                                outs=[cc_out[b].opt()],
                            )
                        else:
                            nc.gpsimd.dma_start(cc_out[b], cc_in[b, 0])
                        if b == 0:
                            nc.gpsimd.dma_start(co0[:], cc_ld[0])
                            # batch-0 FFN1 inputs, on the idle Pool engine
                            for do in range(ND):
                                nc.gpsimd.tensor_add(r1bh0[do][:],
                                                     xgh0[:, do],
                                                     co0[:, do])


                # ---------------- FFN (token-parallel) ----------------
                # Per token-half (h0 = batch-0 chunk, h1 = batch-1 chunk):
                # f1(h) -> f2(h) -> drain. f1h0+f2h0 run on tokens whose
                # ReduceScatter finished mid-attention, so PE stays busy
                # through RS_1 and the co1 load; w2 streams once per half.
                with (
                    tc.tile_pool(name="ldp", bufs=1) as ldp,
                    tc.tile_pool(name="resp", bufs=1) as resp,
                    tc.tile_pool(name="hp", bufs=1) as hp,
                    tc.tile_pool(name="outp", bufs=1) as outp,
                    tc.tile_pool(name="ps_f1", bufs=4, space="PSUM") as ps_f1,
                    tc.tile_pool(name="ps_f2", bufs=1, space="PSUM") as ps_f2,
                ):
                    xgh1 = ldp.tile([P, ND, TOKC], BF16, tag="xgh1")
                    nc.sync.dma_start(xgh1[:], xTg_r[:, :, TOKC:])
                    co1 = ldp.tile([P, ND, TOKC], BF16, tag="co1")
                    nc.gpsimd.dma_start(co1[:], cc_ld[1])
                    res1bh1 = [resp.tile([P, TOKC], BF16, tag=f"r1b_{do}",
                                         name=f"r1b_{do}")
                               for do in range(ND)]
                    res1h = [r1bh0, res1bh1]
                    h_t = [hp.tile([P, 512], BF16, tag=f"h_{fo}",
                                   name=f"h_{fo}") for fo in range(NF)]
                    out_big = outp.tile([P, ND, 512], BF16, tag="o2")
                    # two d-blocks share one PSUM bank (bank granularity)
                    ops = [ps_f2.tile([P, 2, TOKC], F32, tag=f"f2_{dp}",
                                      name=f"f2_{dp}") for dp in range(4)]

                    def emit_f1(half, rhs, interleave=()):
                        # relu+bias on DVE: the Act queue is still draining
                        # attention exps when f1-half0 runs, which would
                        # starve the ps_f1 ring
                        hs = slice(half * TOKC, (half + 1) * TOKC)
                        for fo in range(NF):
                            if fo % 4 == 3 and fo // 4 < len(interleave):
                                interleave[fo // 4]()
                            hps = ps_f1.tile([P, TOKC], F32, tag="f1")
                            fsl = slice(fo * P, (fo + 1) * P)
                            for do in range(ND):
                                nc.tensor.matmul(
                                    hps[:],
                                    w1_t[do // 4][:, do % 4, fsl],
                                    rhs[do][:],
                                    start=(do == 0),
                                    stop=(do == ND - 1))
                            nc.vector.tensor_scalar(
                                h_t[fo][:, hs], hps[:],
                                b1_sb[:, fo:fo + 1], 0.0, OP.add, OP.max)

                    w2_pre = {}

                    def emit_w2_load(key):
                        w2t = w2p.tile([P, 2, D], BF16, tag="w2")
                        fo = key[1]
                        (nc.sync, nc.scalar)[(fo // 2) % 2].dma_start(
                            w2t[:], w2_r[:, fo:fo + 2, :])
                        w2_pre[key] = w2t

                    def emit_f2(half):
                        hs = slice(half * TOKC, (half + 1) * TOKC)
                        for fo in range(NF):
                            if fo % 2 == 0:
                                if (half, fo) not in w2_pre:
                                    emit_w2_load((half, fo))
                                w2t = w2_pre[(half, fo)]
                            for do in range(ND):
                                # PSUM 'start' zeroes the whole bank: only
                                # the even-do chain starts; the odd-do
                                # chain accumulates onto the zeroed bank
                                nc.tensor.matmul(
                                    ops[do // 2][:, do % 2],
                                    w2t[:, fo % 2, do * P:(do + 1) * P],
                                    h_t[fo][:, hs],
                                    start=(fo == 0 and do % 2 == 0),
                                    stop=(fo == NF - 1),
                                    skip_group_check=(do % 2 == 1))

                    def drain_one(half, do):
                        # PSUM reads are DVE-only (GPSIMD cannot touch PSUM)
                        hs = slice(half * TOKC, (half + 1) * TOKC)
                        nc.vector.scalar_tensor_tensor(
                            out_big[:, do, hs], ops[do // 2][:, do % 2],
                            b2_sb[:, do:do + 1],
                            res1h[half][do][:], OP.add, OP.add)

                    emit_f1(0, r1bh0)
                    emit_f2(0)
                    for do in range(ND):
                        nc.vector.tensor_add(res1bh1[do][:],
                                             xgh1[:, do], co1[:, do])
                    emit_w2_load((1, 0))
                    emit_w2_load((1, 2))
                    # half-0 drain interleaves into half-1's relu stream so
                    # the ops banks are free before f2 half-1 needs them
                    emit_f1(1, res1bh1,
                            interleave=[(lambda d=d: drain_one(0, d))
                                        for d in range(ND)])
                    emit_w2_load((1, 4))
                    emit_f2(1)
                    outT_q = outT.rearrange("(q o p) t -> q p o t", q=4, p=P)
                    for do in range(ND):
                        drain_one(1, do)
                        if do % 2 == 1:
                            (nc.sync, nc.scalar)[(do // 2) % 2].dma_start(
                                outT_q[do // 2],
                                out_big[:, do - 1:do + 1])

            for _rep in range(reps):
                emit_body()

    nc.compile()
    return nc


_CACHE = {}


def _sched_key(sched, nm):
    return (tuple((ub, tuple(
        (bt, mi if mi is None else tuple(mi)) for bt, mi in ent))
        for ub, ent in sched), tuple(sorted(nm.items())))


def _build_for(mask, collective=True, reps=1):
    sched, tiles = _mask_schedule(mask)
    nm = {k: len(v) for k, v in tiles.items()}
    key = (_sched_key(sched, nm), collective, reps)
    if key not in _CACHE:
        _CACHE[key] = _build(sched, nm, collective=collective, reps=reps)
    return _CACHE[key]


def prepare_in_maps(encodings, Wq, Wk, Wv, W1, b1, W2, b2, mask):
    x = np.ascontiguousarray(np.asarray(encodings, dtype=np.float32))
    _sched, tiles = _mask_schedule(mask)

    # xTb[b, t, p, o, s] = x[b, t*512+s, o*128+p]
    xTb = np.ascontiguousarray(
        x.reshape(B, 4, 512, ND, P).transpose(0, 1, 4, 3, 2)).astype(BF)
    # x8[b, t, p, dp, j, s] = x[b, t*512+s, (2dp+j)*128+p]
    x8 = np.ascontiguousarray(
        x.reshape(B, 4, 512, ND2, 2, P).transpose(0, 1, 5, 3, 4, 2)
    ).astype(F8)
    w1b = np.ascontiguousarray(
        np.asarray(W1, np.float32).T.reshape(ND, P, DFF)).astype(BF)
    w2b = np.ascontiguousarray(
        np.asarray(W2, np.float32).T.reshape(NF, P, D)).astype(BF)
    b1c = np.ascontiguousarray(np.asarray(b1, np.float32).reshape(NF, P).T)
    b2c = np.ascontiguousarray(np.asarray(b2, np.float32).reshape(ND, P).T)

    def mk(key, w, dt):
        ts = tiles[key]
        if not ts:
            return np.zeros((1, P, w), dt)
        return np.ascontiguousarray(np.stack(ts)).astype(dt)

    mb256 = mk("b256", 256, BF)
    mf256 = mk("f256", 256, F8)
    mf512 = mk("f512", 512, F8)
    onecb = np.ones((P, P), BF)
    one8 = np.ones((P, 2, P), F8)

    in_maps = []
    for c in range(NCORES):
        wvT = np.ascontiguousarray(np.asarray(Wv[c], np.float32).T)  # [d, o]
        # wv8[p, dp, j, o] = wvT[(2dp+j)*128+p, o]
        wv8 = np.ascontiguousarray(
            wvT.reshape(ND2, 2, P, D).transpose(2, 0, 1, 3)).astype(F8)
        xTg = np.ascontiguousarray(np.concatenate(
            [x[0, c * TOKC:(c + 1) * TOKC], x[1, c * TOKC:(c + 1) * TOKC]],
            axis=0).T).astype(BF)
        in_maps.append({
            "xTb": xTb,
            "x8": x8,
            "wqT": np.ascontiguousarray(
                np.asarray(Wq[c], np.float32).T).astype(BF),
            "wkT": np.ascontiguousarray(
                np.asarray(Wk[c], np.float32).T).astype(BF),
            "wv8": wv8,
            "wvTb": wvT.astype(BF),
            "w1b": w1b,
            "w2b": w2b,
            "b1c": b1c,
            "b2c": b2c,
            "mb256": mb256,
            "mf256": mf256,
            "mf512": mf512,
            "onecb": onecb,
            "one8": one8,
            "xTg": xTg,
        })
    return in_maps


def kernel(encodings, Wq, Wk, Wv, W1, b1, W2, b2, mask):
    from concourse.bass_utils import run_bass_kernel_spmd

    nc = _build_for(mask)
    in_maps = prepare_in_maps(encodings, Wq, Wk, Wv, W1, b1, W2, b2, mask)

    res = run_bass_kernel_spmd(nc, in_maps, core_ids=list(range(NCORES)))
    out = np.empty((B, S, D), np.float32)
    for c in range(NCORES):
        o = res.results[c]["outT"]  # [D, 512]
        out[0, c * TOKC:(c + 1) * TOKC] = o[:, :TOKC].T
        out[1, c * TOKC:(c + 1) * TOKC] = o[:, TOKC:].T
    kernel.last_results = res
    return out


# revision 68
# speedup vs baseline: 4.1240x; 4.1240x over previous
"""Trainium2 Bass kernel for an 8-head attention layer + FFN (B=2, S=2048,
D=1024, DQK=128, DFF=4096), distributed over 8 NeuronCores.

Sharding: head-parallel attention (1 head per core), token-parallel FFN.
Each core's FFN tokens are rows [c*256, (c+1)*256) of BOTH batches (512
total). Attention outputs for batch b are stored group-major into
cc_in[b] = [8 groups, D, 256]; ONE ReduceScatter per batch then delivers
each core its own fully-reduced 256-token chunk directly — no AllToAll.
RS_0 fires at 50% attention progress (hidden); RS_1's exposure is hidden
under the first half of FFN1, which only needs batch-0 tokens.

DMAs are batched aggressively (multi-dim access patterns, one DMA per
x-chunk / chunk-store / weight group) since each HWDGE DMA costs ~600ns
of sequencer + DGE time. Loads issue from the two HWDGE queues (sync=SP,
scalar=Act); attention-output stores and collective-adjacent loads go
through gpsimd SWDGE so a store waiting on compute never head-of-line
blocks the x loads, and w1/w2 stream in small pieces that never
monopolize the DMA engines.

Precision plan (validated vs f64 reference in prec_sim.py, ~4e-3 max rel
err vs the 2e-2 gate):
  - q/k projections bf16, qT/kT stored bf16, scores f32 PSUM
  - v projection fp8(e4m3) DoubleRow for all tokens + an extra bf16
    projection for tokens < 256 (first 256 softmax rows are too short to
    average out fp8 noise)
  - softmax: exp in f32->bf16 (rows 0..255) / fp8 (rest); denominators
    from ones-matmuls over the SAME quantized e tiles; no max-subtraction
    (scores are O(1))
  - attnV: bf16 for rows 0..255, fp8 DoubleRow elsewhere
  - FFN bf16 weights/activations, f32 accumulate, bf16 residuals/output
  - collective payload bf16

Score rows are processed in chunks of [256, 256, 512, 512, 512] rows per
batch; each chunk is emitted right after the 512-token projection group
that completes its causal window, so collectives and DMAs hide under
matmul work. DoubleRow tiles carry a k-pair axis (pair outermost, M=128):
x8 [128, 2, 512], v8 [128, 2, D], e8 [128, 2, 512].
"""
import sys

sys.path.insert(0, "/opt/trn_rl_repo")
import numpy as np
import ml_dtypes

B, S, D, H, DQK, DFF = 2, 2048, 1024, 8, 128, 4096
P = 128
TOKC = 256               # tokens per core per batch (FFN sharding)
NG = S // TOKC           # 8 groups per batch
NCORES = 8
NT = S // P              # 16 t-blocks
ND = D // P              # 8 d-blocks
ND2 = ND // 2            # 4 d-block pairs (DoubleRow)
NF = DFF // P            # 32 f-blocks
NVB = 2                  # t-blocks with a bf16 v copy (short rows)
SCALE = 1.0 / float(np.sqrt(DQK))
FFN_BF16 = True          # kept for test.py compat
# (row_start, width) of score-row chunks per batch; chunk 0 runs the bf16
# path. 512-wide chunks store into two 256-token groups.
CHUNKS = [(0, 256), (256, 256), (512, 512), (1024, 512), (1536, 512)]

F8 = ml_dtypes.float8_e4m3
BF = ml_dtypes.bfloat16


def _mask_schedule(mask):
    """Classify each (t-block, chunk) tile of the score matrix.

    Returns (sched, tiles): sched[ci] = (use_bf, entries) where entries is
    a list of (bt, (key, idx) | None); fully-masked tiles are dropped.
    tiles[key] is a list of [P, w] 0/1 fp32 tiles (layout [t, s]) to
    multiply into exp(s); key in {"b256", "f256", "f512"}.
    """
    mask = np.asarray(mask, dtype=bool)
    sched = []
    tiles = {"b256": [], "f256": [], "f512": []}
    uniq = {}
    for ci, (r0, w) in enumerate(CHUNKS):
        entries = []
        for bt in range(NT):
            sub = mask[r0:r0 + w, bt * P:(bt + 1) * P]  # [s, t]
            if sub.all():
                continue
            entries.append((bt, None if not sub.any() else sub))
        bts = [bt for bt, _ in entries]
        use_bf = (ci == 0 or len(bts) % 2 or bts != list(range(len(bts))))
        if use_bf:
            assert w == 256 and all(bt < NVB for bt in bts), \
                "bf16 fallback path requires all entries in t-blocks 0..1"
        key = ("b" if use_bf else "f") + str(w)
        resolved = []
        for bt, sub in entries:
            if sub is None:
                resolved.append((bt, None))
                continue
            tileT = np.where(sub.T, np.float32(0.0), np.float32(1.0)).copy()
            k2 = (key, tileT.tobytes())
            if k2 not in uniq:
                uniq[k2] = len(tiles[key])
                tiles[key].append(tileT)
            resolved.append((bt, (key, uniq[k2])))
        sched.append((use_bf, resolved))
    return sched, tiles


def _build(sched, nm, collective=True, reps=1):
    """nm: dict key->tile count for the three mask-tile tensors."""
    import concourse.mybir as mybir
    import concourse.tile as tile
    from concourse import bacc

    F32 = mybir.dt.float32
    F32R = mybir.dt.float32r
    BF16 = mybir.dt.bfloat16
    FP8 = mybir.dt.float8e4
    AF = mybir.ActivationFunctionType
    OP = mybir.AluOpType
    DR = mybir.MatmulPerfMode.DoubleRow

    nc = bacc.Bacc("TRN2", target_bir_lowering=False, debug=False,
                   num_devices=NCORES)

    xTb_in = nc.dram_tensor("xTb", [B, 4, P, ND, 512], BF16,
                            kind="ExternalInput")
    x8_in = nc.dram_tensor("x8", [B, 4, P, ND2, 2, 512], FP8,
                           kind="ExternalInput")
    wqT_in = nc.dram_tensor("wqT", [D, DQK], BF16, kind="ExternalInput")
    wkT_in = nc.dram_tensor("wkT", [D, DQK], BF16, kind="ExternalInput")
    wv8_in = nc.dram_tensor("wv8", [P, ND2, 2, D], FP8, kind="ExternalInput")
    wvTb_in = nc.dram_tensor("wvTb", [D, D], BF16, kind="ExternalInput")
    w1b_in = nc.dram_tensor("w1b", [ND, P, DFF], BF16, kind="ExternalInput")
    w2b_in = nc.dram_tensor("w2b", [NF, P, D], BF16, kind="ExternalInput")
    b1_in = nc.dram_tensor("b1c", [P, NF], F32, kind="ExternalInput")
    b2_in = nc.dram_tensor("b2c", [P, ND], F32, kind="ExternalInput")
    nb2 = max(nm.get("b256", 0), 1)
    nf2 = max(nm.get("f256", 0), 1)
    nf5 = max(nm.get("f512", 0), 1)
    mb2_in = nc.dram_tensor("mb256", [nb2, P, 256], BF16,
                            kind="ExternalInput")
    mf2_in = nc.dram_tensor("mf256", [nf2, P, 256], FP8,
                            kind="ExternalInput")
    mf5_in = nc.dram_tensor("mf512", [nf5, P, 512], FP8,
                            kind="ExternalInput")
    onecb_in = nc.dram_tensor("onecb", [P, P], BF16, kind="ExternalInput")
    one8_in = nc.dram_tensor("one8", [P, 2, P], FP8, kind="ExternalInput")
    xTg_in = nc.dram_tensor("xTg", [D, 2 * TOKC], BF16,
                        kind="ExternalInput")
    outT = nc.dram_tensor("outT", [D, 2 * TOKC], BF16, kind="ExternalOutput")

    wq_r = wqT_in.rearrange("(o p) e -> p o e", p=P)
    wk_r = wkT_in.rearrange("(o p) e -> p o e", p=P)
    wvb_r = wvTb_in.rearrange("(o p) d -> p o d", p=P)
    w1_r = w1b_in.rearrange("o p f -> p o f")
    w2_r = w2b_in.rearrange("f p d -> p f d")
    mb2_r = mb2_in.rearrange("n p w -> p n w")
    mf2_r = mf2_in.rearrange("n p w -> p n w")
    mf5_r = mf5_in.rearrange("n p w -> p n w")
    xTg_r = xTg_in.rearrange("(o p) t -> p o t", p=P)
    outT_r = outT.rearrange("(h o p) t -> h p o t", h=2, p=P)

    with tile.TileContext(nc) as tc:
        with (
            tc.tile_pool(name="consts", bufs=1) as consts,
            tc.tile_pool(name="w2p", bufs=4) as w2p,
            tc.tile_pool(name="dram", bufs=1, space="DRAM") as dram,
        ):
            # const tiles are allocated here but their load DMAs are emitted
            # inside emit_body right after the first x loads, ordered by
            # first use, so the critical first projection isn't queued
            # behind ~6MB of constants on the shared HWDGE.
            ones_cb = consts.tile([P, P], BF16, tag="onecb")
            ones8 = consts.tile([P, 2, P], FP8, tag="one8")
            b1_sb = consts.tile([P, NF], F32, tag="b1")
            b2_sb = consts.tile([P, ND], F32, tag="b2")
            mb2_sb = consts.tile([P, nb2, 256], BF16, tag="mb2")
            mf2_sb = consts.tile([P, nf2, 256], FP8, tag="mf2")
            mf5_sb = consts.tile([P, nf5, 512], FP8, tag="mf5")
            msk_sb = {"b256": mb2_sb, "f256": mf2_sb, "f512": mf5_sb}
            wq_all = consts.tile([P, ND, DQK], BF16, tag="wq", name="wq")
            wk_all = consts.tile([P, ND, DQK], BF16, tag="wk", name="wk")
            wv8_all = consts.tile([P, ND2, 2, D], FP8, tag="wv8", name="wv8")
            wvb_all = consts.tile([P, ND, D], BF16, tag="wvb", name="wvb")
            w1_t = [consts.tile([P, ND2, DFF], BF16, tag=f"w1_{i}",
                                name=f"w1_{i}") for i in range(2)]
            # batch-0 FFN inputs, prepared during attention (gpsimd SWDGE
            # keeps the loads off the two busy HWDGE queues) so FFN1
            # half-0 can start the instant attention ends and hide RS_1
            # under matmul work
            xgh0 = consts.tile([P, ND, TOKC], BF16, tag="xgh0")
            co0 = consts.tile([P, ND, TOKC], BF16, tag="co0")
            r1bh0 = [consts.tile([P, TOKC], BF16, tag=f"r1bh0_{do}",
                                 name=f"r1bh0_{do}") for do in range(ND)]
            w1_loaded = []
            consts_loaded = []

            def emit_const_loads():
                nc.sync.dma_start(ones_cb[:], onecb_in[:])
                nc.scalar.dma_start(mb2_sb[:], mb2_r[:])
                nc.sync.dma_start(mf2_sb[:], mf2_r[:])
                nc.scalar.dma_start(mf5_sb[:], mf5_r[:])
                nc.sync.dma_start(ones8[:], one8_in[:])
                nc.scalar.dma_start(b1_sb[:], b1_in[:])
                nc.sync.dma_start(b2_sb[:], b2_in[:])
                nc.scalar.dma_start(wvb_all[:], wvb_r[:])
                nc.sync.dma_start(wv8_all[:], wv8_in[:])

            def emit_body():
                cc_in = dram.tile([B, NG, D, TOKC], BF16, tag="cc_in",
                                  name="cc_in")
                cc_out = dram.tile([B, D, TOKC], BF16, tag="cc_out",
                                   name="cc_out")
                cc_st1 = cc_in.rearrange("b g (o p) s -> b g p o s", p=P)
                cc_ld = cc_out.rearrange("b (o p) s -> b p o s", p=P)

                # ---------------- attention (head-parallel) ----------------
                with (
                    tc.tile_pool(name="xt", bufs=2) as xtp,
                    tc.tile_pool(name="qk", bufs=1) as qkp,
                    tc.tile_pool(name="vp", bufs=1) as vp,
                    tc.tile_pool(name="ep", bufs=10) as ep,
                    tc.tile_pool(name="ebp", bufs=4) as ebp,
                    tc.tile_pool(name="rbp", bufs=2) as rbp,
                    tc.tile_pool(name="aop", bufs=3) as aop,
                    tc.tile_pool(name="ps_pr", bufs=2, space="PSUM") as ps_pr,
                    tc.tile_pool(name="ps_sc", bufs=2, space="PSUM") as ps_sc,
                    tc.tile_pool(name="ps_sum", bufs=1, space="PSUM") as ps_sum,
                    tc.tile_pool(name="ps_at", bufs=3, space="PSUM") as ps_at,
                ):
                    nc.gpsimd.dma_start(xgh0[:], xTg_r[:, :, :TOKC])
                    for b in range(B):
                        qT_t = [qkp.tile([P, 512], BF16, tag=f"qT{t}",
                                         name=f"qT{t}") for t in range(4)]
                        kT_t = [qkp.tile([P, 512], BF16, tag=f"kT{t}",
                                         name=f"kT{t}") for t in range(4)]
                        v8_t = [vp.tile([P, 2, D], FP8, tag=f"v8_{m}",
                                        name=f"v8_{m}")
                                for m in range(NT // 2)]
                        vb_t = [vp.tile([P, D], BF16, tag=f"vb{ti}",
                                        name=f"vb{ti}") for ti in range(NVB)]

                        def emit_chunk(ci):
                            r0, w = CHUNKS[ci]
                            use_bf, entries = sched[ci]
                            if not entries:
                                return
                            tq = r0 // 512
                            qsl = slice(r0 % 512, r0 % 512 + w)
                            sums = ps_sum.tile([P, 512], F32, tag="sum")
                            ao_t = [aop.tile([P, ND, TOKC], BF16, tag="ao",
                                             name=f"ao{gi}")
                                    for gi in range(w // TOKC)]
                            e_sb = {}
                            e_pr = []
                            npair = len(entries) // 2
                            if not use_bf:
                                e_pr = [ep.tile([P, 2, 512], FP8, tag="e8",
                                                name=f"e8_{m}")
                                        for m in range(npair)]
                            # scores, exp, and the ones-matmul denominators
                            # interleaved so the sums never wait on the
                            # full exp backlog
                            for i, (bt, mi) in enumerate(entries):
                                sp = ps_sc.tile([P, 512], F32, tag="sc")
                                nc.tensor.matmul(
                                    sp[:, :w],
                                    kT_t[bt // 4][:, (bt % 4) * P:
                                                  (bt % 4 + 1) * P],
                                    qT_t[tq][:, qsl],
                                    start=True, stop=True)
                                if use_bf:
                                    eb = ebp.tile([P, 256], BF16, tag="eb")
                                    e_sb[bt] = eb
                                    ev = eb[:]
                                else:
                                    ev = e_pr[i // 2][:, i % 2, :w]
                                nc.scalar.activation(ev, sp[:, :w],
                                                     AF.Exp, scale=SCALE)
                                if mi is not None:
                                    nc.vector.tensor_tensor(
                                        ev, ev, msk_sb[mi[0]][:, mi[1], :],
                                        OP.mult)
                                if use_bf:
                                    nc.tensor.matmul(
                                        sums[:, :w], ones_cb[:], ev,
                                        start=(i == 0),
                                        stop=(i == len(entries) - 1))
                                elif i % 2 == 1:
                                    nc.tensor.matmul(
                                        sums[:, :w], ones8[:],
                                        e_pr[i // 2][:, :, :w],
                                        start=(i == 1),
                                        stop=(i == len(entries) - 1),
                                        perf_mode=DR)
                            rb = rbp.tile([P, 512], F32R, tag="rb")
                            with nc.allow_low_precision(
                                    reason="softmax 1/sum in f32r"):
                                nc.vector.reciprocal(rb[:, :w], sums[:, :w])
                            # attnV pair-outer over 4-oc halves: each mm
                            # only needs the next exp pair, so PE chews
                            # through attnV while later exps still run
                            for ocs in ([0], [1], [2], [3], [4], [5],
                                        [6], [7]):
                                aps = {oc: ps_at.tile([P, 512], F32,
                                                      tag="at",
                                                      name=f"at{oc}")
                                       for oc in ocs}
                                if use_bf:
                                    for i, (bt, _mi) in enumerate(entries):
                                        for oc in ocs:
                                            nc.tensor.matmul(
                                                aps[oc][:, :w],
                                                vb_t[bt][:, oc * P:
                                                         (oc + 1) * P],
                                                e_sb[bt][:],
                                                start=(i == 0),
                                                stop=(i == len(entries)
                                                      - 1))
                                else:
                                    for m in range(npair):
                                        for oc in ocs:
                                            nc.tensor.matmul(
                                                aps[oc][:, :w],
                                                v8_t[m][:, :, oc * P:
                                                        (oc + 1) * P],
                                                e_pr[m][:, :, :w],
                                                start=(m == 0),
                                                stop=(m == npair - 1),
                                                perf_mode=DR)
                                for oc in ocs:
                                    for gi in range(w // TOKC):
                                        gs = slice(gi * TOKC,
                                                   (gi + 1) * TOKC)
                                        nc.vector.tensor_tensor(
                                            ao_t[gi][:, oc], aps[oc][:, gs],
                                            rb[:, gs], OP.mult)
                            # stores go through gpsimd SWDGE: the Pool
                            # queue is otherwise idle, and a store waiting
                            # on attention output would head-of-line block
                            # the x loads behind it on a HWDGE queue
                            g0 = r0 // TOKC
                            for gi in range(w // TOKC):
                                nc.gpsimd.dma_start(cc_st1[b, g0 + gi],
                                                    ao_t[gi][:])

                        for tch in range(4):  # 512-token chunks of S
                            if not consts_loaded:
                                # tiny q/k weights first so the first
                                # projection isn't queued behind x
                                nc.sync.dma_start(wq_all[:], wq_r[:])
                                nc.scalar.dma_start(wk_all[:], wk_r[:])
                            xb_all = xtp.tile([P, ND, 512], BF16, tag="xb")
                            nc.sync.dma_start(xb_all[:], xTb_in[b, tch])
                            x8_all = xtp.tile([P, ND2, 2, 512], FP8,
                                              tag="x8")
                            nc.scalar.dma_start(x8_all[:], x8_in[b, tch])
                            if not consts_loaded:
                                consts_loaded.append(True)
                                emit_const_loads()
                            qps = ps_pr.tile([P, 512], F32, tag="pr")
                            for do in range(ND):
                                nc.tensor.matmul(qps[:], wq_all[:, do],
                                                 xb_all[:, do],
                                                 start=(do == 0),
                                                 stop=(do == ND - 1))
                            nc.vector.tensor_copy(qT_t[tch][:], qps[:])
                            kps = ps_pr.tile([P, 512], F32, tag="pr")
                            for do in range(ND):
                                nc.tensor.matmul(kps[:], wk_all[:, do],
                                                 xb_all[:, do],
                                                 start=(do == 0),
                                                 stop=(do == ND - 1))
                            nc.vector.tensor_copy(kT_t[tch][:], kps[:])
                            for ti in range(4):  # t-blocks within this chunk
                                to = tch * 4 + ti
                                tsl = slice(ti * P, (ti + 1) * P)
                                for oc in range(2):
                                    osl = slice(oc * 512, (oc + 1) * 512)
                                    vps = ps_pr.tile([P, 512], F32, tag="pr")
                                    for dp in range(ND2):
                                        nc.tensor.matmul(
                                            vps[:], x8_all[:, dp, :, tsl],
                                            wv8_all[:, dp, :, osl],
                                            start=(dp == 0),
                                            stop=(dp == ND2 - 1),
                                            perf_mode=DR)
                                    nc.vector.tensor_copy(
                                        v8_t[to // 2][:, to % 2, osl],
                                        vps[:])
                                    if tch == 0 and ti < NVB:
                                        vbs = ps_pr.tile([P, 512], F32,
                                                         tag="pr")
                                        for do in range(ND):
                                            nc.tensor.matmul(
                                                vbs[:],
                                                xb_all[:, do, tsl],
                                                wvb_all[:, do, osl],
                                                start=(do == 0),
                                                stop=(do == ND - 1))
                                        nc.scalar.copy(vb_t[ti][:, osl],
                                                       vbs[:])
                            # w1 streams in 1MB pieces (2 per 512-token
                            # chunk) so it never monopolizes the DMA
                            # engines ahead of the latency-critical x loads
                            if not (b == 0 and tch == 0) and \
                                    len(w1_loaded) < ND:
                                n = 2 if len(w1_loaded) < 2 else 1
                                for _ in range(n):
                                    i = len(w1_loaded)
                                    w1_loaded.append(True)
                                    (nc.scalar, nc.sync)[i % 2].dma_start(
                                        w1_t[i // 4][:, i % 4:i % 4 + 1, :],
                                        w1_r[:, i:i + 1, :])
                            if tch == 0:
                                emit_chunk(0)
                                emit_chunk(1)
                            else:
                                emit_chunk(tch + 1)
                        if collective:
                            nc.gpsimd.collective_compute(
                                "ReduceScatter",
                                mybir.AluOpType.add,
                                replica_groups=[list(range(NCORES))],
                                ins=[cc_in[b].opt()],
                                outs=[cc_out[b].opt()],
                            )
                        else:
                            nc.gpsimd.dma_start(cc_out[b], cc_in[b, 0])
                        if b == 0:
                            nc.gpsimd.dma_start(co0[:], cc_ld[0])
                            # batch-0 FFN1 inputs, on the idle Pool engine
                            for do in range(ND):
                                nc.gpsimd.tensor_add(r1bh0[do][:],
                                                     xgh0[:, do],
                                                     co0[:, do])


                # ---------------- FFN (token-parallel) ----------------
                # Per token-half (h0 = batch-0 chunk, h1 = batch-1 chunk):
                # f1(h) -> f2(h) -> drain. f1h0+f2h0 run on tokens whose
                # ReduceScatter finished mid-attention, so PE stays busy
                # through RS_1 and the co1 load; w2 streams once per half.
                with (
                    tc.tile_pool(name="ldp", bufs=1) as ldp,
                    tc.tile_pool(name="resp", bufs=1) as resp,
                    tc.tile_pool(name="hp", bufs=1) as hp,
                    tc.tile_pool(name="outp", bufs=1) as outp,
                    tc.tile_pool(name="ps_f1", bufs=4, space="PSUM") as ps_f1,
                    tc.tile_pool(name="ps_f2", bufs=1, space="PSUM") as ps_f2,
                ):
                    xgh1 = ldp.tile([P, ND, TOKC], BF16, tag="xgh1")
                    nc.sync.dma_start(xgh1[:], xTg_r[:, :, TOKC:])
                    co1 = ldp.tile([P, ND, TOKC], BF16, tag="co1")
                    nc.gpsimd.dma_start(co1[:], cc_ld[1])
                    res1bh1 = [resp.tile([P, TOKC], BF16, tag=f"r1b_{do}",
                                         name=f"r1b_{do}")
                               for do in range(ND)]
                    res1h = [r1bh0, res1bh1]
                    h_t = [hp.tile([P, 512], BF16, tag=f"h_{fo}",
                                   name=f"h_{fo}") for fo in range(NF)]
                    out_big = outp.tile([P, ND, 512], BF16, tag="o2")
                    # two d-blocks share one PSUM bank (bank granularity)
                    ops = [ps_f2.tile([P, 2, TOKC], F32, tag=f"f2_{dp}",
                                      name=f"f2_{dp}") for dp in range(4)]

                    def emit_f1(half, rhs, interleave=()):
                        # relu+bias on DVE: the Act queue is still draining
                        # attention exps when f1-half0 runs, which would
                        # starve the ps_f1 ring
                        hs = slice(half * TOKC, (half + 1) * TOKC)
                        for fo in range(NF):
                            if fo % 4 == 3 and fo // 4 < len(interleave):
                                interleave[fo // 4]()
                            hps = ps_f1.tile([P, TOKC], F32, tag="f1")
                            fsl = slice(fo * P, (fo + 1) * P)
                            for do in range(ND):
                                nc.tensor.matmul(
                                    hps[:],
                                    w1_t[do // 4][:, do % 4, fsl],
                                    rhs[do][:],
                                    start=(do == 0),
                                    stop=(do == ND - 1))
                            nc.vector.tensor_scalar(
                                h_t[fo][:, hs], hps[:],
                                b1_sb[:, fo:fo + 1], 0.0, OP.add, OP.max)

                    w2_pre = {}

                    def emit_w2_load(key):
                        w2t = w2p.tile([P, 2, D], BF16, tag="w2")
                        fo = key[1]
                        (nc.sync, nc.scalar)[(fo // 2) % 2].dma_start(
                            w2t[:], w2_r[:, fo:fo + 2, :])
                        w2_pre[key] = w2t

                    def emit_f2(half):
                        hs = slice(half * TOKC, (half + 1) * TOKC)
                        for fo in range(NF):
                            if fo % 2 == 0:
                                if (half, fo) not in w2_pre:
                                    emit_w2_load((half, fo))
                                w2t = w2_pre[(half, fo)]
                            for do in range(ND):
                                # PSUM 'start' zeroes the whole bank: only
                                # the even-do chain starts; the odd-do
                                # chain accumulates onto the zeroed bank
                                nc.tensor.matmul(
                                    ops[do // 2][:, do % 2],
                                    w2t[:, fo % 2, do * P:(do + 1) * P],
                                    h_t[fo][:, hs],
                                    start=(fo == 0 and do % 2 == 0),
                                    stop=(fo == NF - 1),
                                    skip_group_check=(do % 2 == 1))

                    def drain_one(half, do):
                        # PSUM reads are DVE-only (GPSIMD cannot touch PSUM)
                        hs = slice(half * TOKC, (half + 1) * TOKC)
                        nc.vector.scalar_tensor_tensor(
                            out_big[:, do, hs], ops[do // 2][:, do % 2],
                            b2_sb[:, do:do + 1],
                            res1h[half][do][:], OP.add, OP.add)

                    emit_f1(0, r1bh0)
                    emit_f2(0)
                    for do in range(ND):
                        nc.vector.tensor_add(res1bh1[do][:],
                                             xgh1[:, do], co1[:, do])
                    emit_w2_load((1, 0))
                    emit_w2_load((1, 2))
                    # half-0 drain interleaves into half-1's relu stream so
                    # the ops banks are free before f2 half-1 needs them
                    emit_f1(1, res1bh1,
                            interleave=[(lambda d=d: drain_one(0, d))
                                        for d in range(ND)])
                    emit_w2_load((1, 4))
                    emit_f2(1)
                    outT_q = outT.rearrange("(q o p) t -> q p o t", q=4, p=P)
                    for do in range(ND):
                        drain_one(1, do)
                        if do % 2 == 1:
                            (nc.sync, nc.scalar)[(do // 2) % 2].dma_start(
                                outT_q[do // 2],
                                out_big[:, do - 1:do + 1])

            for _rep in range(reps):
                emit_body()

    nc.compile()
    return nc


_CACHE = {}


def _sched_key(sched, nm):
    return (tuple((ub, tuple(
        (bt, mi if mi is None else tuple(mi)) for bt, mi in ent))
        for ub, ent in sched), tuple(sorted(nm.items())))


def _build_for(mask, collective=True, reps=1):
    sched, tiles = _mask_schedule(mask)
    nm = {k: len(v) for k, v in tiles.items()}
    key = (_sched_key(sched, nm), collective, reps)
    if key not in _CACHE:
        _CACHE[key] = _build(sched, nm, collective=collective, reps=reps)
    return _CACHE[key]


def prepare_in_maps(encodings, Wq, Wk, Wv, W1, b1, W2, b2, mask):
    x = np.ascontiguousarray(np.asarray(encodings, dtype=np.float32))
    _sched, tiles = _mask_schedule(mask)

    # xTb[b, t, p, o, s] = x[b, t*512+s, o*128+p]
    xTb = np.ascontiguousarray(
        x.reshape(B, 4, 512, ND, P).transpose(0, 1, 4, 3, 2)).astype(BF)
    # x8[b, t, p, dp, j, s] = x[b, t*512+s, (2dp+j)*128+p]
    x8 = np.ascontiguousarray(
        x.reshape(B, 4, 512, ND2, 2, P).transpose(0, 1, 5, 3, 4, 2)
    ).astype(F8)
    w1b = np.ascontiguousarray(
        np.asarray(W1, np.float32).T.reshape(ND, P, DFF)).astype(BF)
    w2b = np.ascontiguousarray(
        np.asarray(W2, np.float32).T.reshape(NF, P, D)).astype(BF)
    b1c = np.ascontiguousarray(np.asarray(b1, np.float32).reshape(NF, P).T)
    b2c = np.ascontiguousarray(np.asarray(b2, np.float32).reshape(ND, P).T)

    def mk(key, w, dt):
        ts = tiles[key]
        if not ts:
            return np.zeros((1, P, w), dt)
        return np.ascontiguousarray(np.stack(ts)).astype(dt)

    mb256 = mk("b256", 256, BF)
    mf256 = mk("f256", 256, F8)
    mf512 = mk("f512", 512, F8)
    onecb = np.ones((P, P), BF)
    one8 = np.ones((P, 2, P), F8)

    in_maps = []
    for c in range(NCORES):
        wvT = np.ascontiguousarray(np.asarray(Wv[c], np.float32).T)  # [d, o]
        # wv8[p, dp, j, o] = wvT[(2dp+j)*128+p, o]
        wv8 = np.ascontiguousarray(
            wvT.reshape(ND2, 2, P, D).transpose(2, 0, 1, 3)).astype(F8)
        xTg = np.ascontiguousarray(np.concatenate(
            [x[0, c * TOKC:(c + 1) * TOKC], x[1, c * TOKC:(c + 1) * TOKC]],
            axis=0).T).astype(BF)
        in_maps.append({
            "xTb": xTb,
            "x8": x8,
            "wqT": np.ascontiguousarray(
                np.asarray(Wq[c], np.float32).T).astype(BF),
            "wkT": np.ascontiguousarray(
                np.asarray(Wk[c], np.float32).T).astype(BF),
            "wv8": wv8,
            "wvTb": wvT.astype(BF),
            "w1b": w1b,
            "w2b": w2b,
            "b1c": b1c,
            "b2c": b2c,
            "mb256": mb256,
            "mf256": mf256,
            "mf512": mf512,
            "onecb": onecb,
            "one8": one8,
            "xTg": xTg,
        })
    return in_maps


def kernel(encodings, Wq, Wk, Wv, W1, b1, W2, b2, mask):
    from concourse.bass_utils import run_bass_kernel_spmd

    nc = _build_for(mask)
    in_maps = prepare_in_maps(encodings, Wq, Wk, Wv, W1, b1, W2, b2, mask)

    res = run_bass_kernel_spmd(nc, in_maps, core_ids=list(range(NCORES)))
    out = np.empty((B, S, D), np.float32)
    for c in range(NCORES):
        o = res.results[c]["outT"]  # [D, 512]
        out[0, c * TOKC:(c + 1) * TOKC] = o[:, :TOKC].T
        out[1, c * TOKC:(c + 1) * TOKC] = o[:, TOKC:].T
    kernel.last_results = res
    return out
